# revision 16
# baseline (speedup 1.0000x reference)
"""Trainium2 Bass kernel for the AttentionCritic problem.

Strategy (pure data-parallel over batch, 8 cores), V2:
  - Host: transpose states/actions to feature-major, cast to bf16, pack
    per-head weights into merged [128,128] matrices, precompute the
    argmax one-hot selector on host, build small 0/1 selector matrices
    for PE-based partition reductions/broadcasts.
  - Device (per core, batch shard 4096, feature-major layout
    [feature_on_partitions, batch_on_free]):
      * dense encoders / K,Q,V / critic on TensorE (bf16), n-outer over
        superchunks so weight loads (LDWEIGHTS) amortize,
      * logits for 4 agents packed per PSUM bank; L-reduce uses one
        shared ones[128,4] lhsT writing 4-row regions (one LDW total),
      * exp on ScalarE over packed [128,S] banks (2 per chunk),
      * denominators for all 8 agents in one [32,S] bank; one
        reciprocal_approx_fast; normalization multiplier broadcast via
        one fp32r matmul per bank; A = E*M on Pool,
      * numerator: per (i,j) broadcast matmul (selbc_j lhsT, j-outer so
        LDW amortizes) -> PSUM, multiply with V_j on Pool/Vector
        reading PSUM directly -> bf16 SBUF product,
      * j-accumulation fused into the critic: h1_i = wc1a_i@senc_i +
        sum_j wc1b_i@(A_ij*V_j) accumulated in PSUM (linearity),
      * critic head relu / output bias on ScalarE, DMA out.
"""

import sys

sys.path.insert(0, "/opt/trn_rl_repo")

import numpy as np
import ml_dtypes

N, B, SDIM, ADIM, HID, HEADS = 8, 32768, 128, 16, 128, 4
AD = HID // HEADS
IDIM = SDIM + ADIM
NCORES = 8
BSH = B // NCORES
BF16 = ml_dtypes.bfloat16
INV_SQRT_AD = 1.0 / np.sqrt(AD).astype(np.float32)


def build_nc(bsh, SC=1024, CS=512, split=True):
    """Build the Bass module for one core processing a batch shard of bsh.

    SC: superchunk (dense n-outer granularity; senc/K/Q/V persist per SC)
    CS: chunk (PSUM bank free size; all attention tiles are [*, CS])
    """
    import concourse.bass as bass
    import concourse.mybir as mybir
    from concourse.tile import TileContext

    f32 = mybir.dt.float32
    f32r = mybir.dt.float32r
    bf16 = mybir.dt.bfloat16
    MULT = mybir.AluOpType.mult
    ADD = mybir.AluOpType.add
    MAX = mybir.AluOpType.max
    COPY = mybir.ActivationFunctionType.Copy
    RELU = mybir.ActivationFunctionType.Relu
    EXP = mybir.ActivationFunctionType.Exp
    IDENT = mybir.ActivationFunctionType.Identity

    SC = min(SC, bsh)
    CS = min(CS, SC)
    n_sc = bsh // SC
    n_cs = SC // CS

    nc = bass.Bass()

    # ---- DRAM parameters ----
    dp = nc.declare_dram_parameter
    sT = dp("sT", [N, SDIM, bsh], bf16, isOutput=False)
    aT = dp("aT", [N, ADIM, bsh], bf16, isOutput=False)
    we1 = dp("we1", [N, SDIM, HID], bf16, isOutput=False)
    we2 = dp("we2", [N, ADIM, HID], bf16, isOutput=False)
    ws = dp("ws", [N, SDIM, HID], bf16, isOutput=False)
    wk = dp("wk", [HID, HID], bf16, isOutput=False)
    wq = dp("wq", [HID, HID], bf16, isOutput=False)
    wv = dp("wv", [HID, HID], bf16, isOutput=False)
    wc1a = dp("wc1a", [N, HID, HID], bf16, isOutput=False)
    wc1b = dp("wc1b", [N, HID, HID], bf16, isOutput=False)
    wc2 = dp("wc2", [N, HID, ADIM], bf16, isOutput=False)
    onesred = dp("onesred", [HID, N * 32], bf16, isOutput=False)  # per-j L-reduce
    seld32 = dp("seld32", [N, HID, 32], bf16, isOutput=False)  # E(j,k),j!=i -> k
    selrep2 = dp("selrep2", [HID, HID], bf16, isOutput=False)  # Dinv(g,k)->(g,j,k)
    selbc = dp("selbc", [HID, N * HID], bf16, isOutput=False)  # (j,k)->(k,d)
    bE = dp("bE", [N, HID, 1], f32, isOutput=False)
    bS = dp("bS", [N, HID, 1], f32, isOutput=False)
    bV = dp("bV", [HID, 1], f32, isOutput=False)
    bC1 = dp("bC1", [N, HID, 1], f32, isOutput=False)
    bC2 = dp("bC2", [N, ADIM, 1], f32, isOutput=False)
    q8 = dp("q8", [N, ADIM, bsh], f32, isOutput=True)

    with TileContext(nc) as tc:
        with tc.tile_pool(name="const", bufs=1) as cp:

            def cload(ap, shape, dt, tag):
                t = cp.tile(shape, dt, tag=tag)
                nc.sync.dma_start(out=t[:], in_=ap)
                return t

            we1_s = [cload(we1[n], [SDIM, HID], bf16, f"we1{n}") for n in range(N)]
            we2_s = [cload(we2[n], [ADIM, HID], bf16, f"we2{n}") for n in range(N)]
            ws_s = [cload(ws[n], [SDIM, HID], bf16, f"ws{n}") for n in range(N)]
            wk_s = cload(wk[:], [HID, HID], bf16, "wk")
            wq_s = cload(wq[:], [HID, HID], bf16, "wq")
            wv_s = cload(wv[:], [HID, HID], bf16, "wv")
            wc1a_s = [cload(wc1a[n], [HID, HID], bf16, f"wc1a{n}") for n in range(N)]
            wc1b_s = [cload(wc1b[n], [HID, HID], bf16, f"wc1b{n}") for n in range(N)]
            wc2_s = [cload(wc2[n], [HID, ADIM], bf16, f"wc2{n}") for n in range(N)]
            onesred_s = cload(onesred[:], [HID, N * 32], bf16, "onesred")
            seld32_s = [cload(seld32[n], [HID, 32], bf16, f"seld32{n}") for n in range(N)]
            selrep2_s = cload(selrep2[:], [HID, HID], bf16, "selrep2")
            selbc_s = cload(selbc[:], [HID, N * HID], bf16, "selbc")
            bE_s = [cload(bE[n], [HID, 1], f32, f"bE{n}") for n in range(N)]
            bS_s = [cload(bS[n], [HID, 1], f32, f"bS{n}") for n in range(N)]
            bV_s = cload(bV[:], [HID, 1], f32, "bV")
            bC1_s = [cload(bC1[n], [HID, 1], f32, f"bC1{n}") for n in range(N)]
            bC2_s = [cload(bC2[n], [ADIM, 1], f32, f"bC2{n}") for n in range(N)]

            with (
                tc.tile_pool(name="persist", bufs=1) as pp,
                tc.tile_pool(name="work", bufs=2) as wp,
                tc.tile_pool(name="prp", bufs=8) as prp,
                tc.tile_pool(name="psum", bufs=1, space="PSUM") as qp,
            ):
                # 8 PSUM banks, one tag each.  Dense phase reuses them so
                # every region is written (start=True) before being read.
                def bank(tag):
                    return qp.tile([HID, CS], f32, tag=tag, name=tag)

                BTAGS = ["Lp0", "Lp1", "scr0", "scr1", "h0", "h1", "h2", "h3"]

                for sc in range(n_sc):
                    sc0 = sc * SC
                    senc = [pp.tile([HID, SC], bf16, tag=f"senc{n}", name=f"senc{n}") for n in range(N)]
                    Kt = [pp.tile([HID, SC], bf16, tag=f"K{n}", name=f"K{n}") for n in range(N)]
                    Qt = [pp.tile([HID, SC], bf16, tag=f"Q{n}", name=f"Q{n}") for n in range(N)]
                    Vt = [pp.tile([HID, SC], bf16, tag=f"V{n}", name=f"V{n}") for n in range(N)]

                    # ---------- dense phase (n-outer, LDW amortized) ------
                    for n in range(N):
                        for c in range(n_cs):
                            c0 = sc0 + c * CS
                            sl = slice(c * CS, (c + 1) * CS)
                            st = wp.tile([SDIM, CS], bf16, tag="st", name="st")
                            at = wp.tile([ADIM, CS], bf16, tag="at", name="at")
                            nc.sync.dma_start(out=st[:], in_=sT[n, :, c0 : c0 + CS])
                            nc.sync.dma_start(out=at[:], in_=aT[n, :, c0 : c0 + CS])
                            pe_ = bank(BTAGS[(2 * c) % 8])
                            po_ = bank(BTAGS[(2 * c + 1) % 8])
                            # saenc = relu(we1@st + we2@at + bE)
                            nc.tensor.matmul(pe_[:], we1_s[n][:], st[:], start=True, stop=False)
                            nc.tensor.matmul(pe_[:], we2_s[n][:], at[:], start=False, stop=True)
                            saenc = wp.tile([HID, CS], bf16, tag="saenc", name="saenc")
                            nc.vector.tensor_scalar(saenc[:], pe_[:], bE_s[n][:], 0.0, ADD, MAX)
                            # senc = relu(ws@st + bS)
                            nc.tensor.matmul(po_[:], ws_s[n][:], st[:], start=True, stop=True)
                            nc.vector.tensor_scalar(senc[n][:, sl], po_[:], bS_s[n][:], 0.0, ADD, MAX)
                            # K = wk@saenc ; Q = wq@senc ; V = relu(wv@saenc+bV)
                            pk = bank(BTAGS[(2 * c + 2) % 8])
                            nc.tensor.matmul(pk[:], wk_s[:], saenc[:], start=True, stop=True)
                            nc.vector.tensor_copy(Kt[n][:, sl], pk[:])
                            pq = bank(BTAGS[(2 * c + 3) % 8])
                            nc.tensor.matmul(pq[:], wq_s[:], senc[n][:, sl], start=True, stop=True)
                            nc.scalar.activation(Qt[n][:, sl], pq[:], COPY)
                            pv = bank(BTAGS[(2 * c + 4) % 8])
                            nc.tensor.matmul(pv[:], wv_s[:], saenc[:], start=True, stop=True)
                            nc.scalar.activation(Vt[n][:, sl], pv[:], RELU, bias=bV_s[:])

                    # ---------- attention + critic, per chunk -------------
                    for c in range(n_cs):
                        c0 = sc0 + c * CS
                        sl = slice(c * CS, (c + 1) * CS)
                        Lp = [bank("Lp0"), bank("Lp1")]
                        # logits: pair (i,j) accumulates rows (4j+k) of the
                        # 32-row region 32*(i%4) of bank i//4; self rows stay
                        # exactly zero (every lhsT slice is 0 there)
                        for j in range(N):
                            for i in range(N):
                                if i == j:
                                    continue
                                pr = prp.tile([HID, CS], bf16, tag="pr", name="pr")
                                nc.vector.tensor_tensor(pr[:], Qt[i][:, sl], Kt[j][:, sl], MULT)
                                g = i % 4
                                nc.tensor.matmul(
                                    Lp[i // 4][32 * g : 32 * g + 32, :],
                                    onesred_s[:, 32 * j : 32 * j + 32],
                                    pr[:],
                                    start=(j == (1 if i == 0 else 0)),
                                    stop=(j == (N - 2 if i == N - 1 else N - 1)),
                                    tile_position=(0, 32 * g),
                                    skip_group_check=True,
                                )
                        # E = exp(L/sqrt(d)) (bf16); self rows exp(0)=1, excluded by seld
                        Et = [wp.tile([HID, CS], bf16, tag=f"E{b}", name=f"E{b}") for b in range(2)]
                        for b in range(2):
                            nc.scalar.activation(Et[b][:], Lp[b][:], EXP, scale=float(INV_SQRT_AD))
                        # denominators: D_i head k at row 32*(i%4)+k of bank i//4
                        # (cols 4..31 of seld32 are all-ones fillers so the rest
                        # of each region holds finite positive sums)
                        Dp = [bank("scr0"), bank("scr1")]
                        for i in range(N):
                            g = i % 4
                            nc.tensor.matmul(
                                Dp[i // 4][32 * g : 32 * g + 32, :],
                                seld32_s[i][32 * g : 32 * g + 32, :],
                                Et[i // 4][32 * g : 32 * g + 32, :],
                                start=True,
                                stop=True,
                                tile_position=(32 * g, 32 * g),
                                skip_group_check=True,
                            )
                        Dinv = [wp.tile([HID, CS], f32, tag=f"Dinv{b}", name=f"Dinv{b}") for b in range(2)]
                        Dinvb = [wp.tile([HID, CS], bf16, tag=f"Dinvb{b}", name=f"Dinvb{b}") for b in range(2)]
                        for b in range(2):
                            nc.vector.reciprocal(Dinv[b][:], Dp[b][:])
                            nc.vector.tensor_copy(Dinvb[b][:], Dinv[b][:])
                        # A = E * broadcast(Dinv) per bank (fp32r matmul + Pool)
                        At = [wp.tile([HID, CS], bf16, tag=f"A{b}", name=f"A{b}") for b in range(2)]
                        for b in range(2):
                            Mp = bank("scr1" if b == 0 else "scr0")
                            nc.tensor.matmul(
                                Mp[:],
                                selrep2_s[:],
                                Dinvb[b][:],
                                start=True,
                                stop=True,
                            )
                            nc.vector.tensor_tensor(At[b][:], Et[b][:], Mp[:], MULT)
                        # numerator + critic h1, i-halves x j-outer
                        scnt = [0]
                        SCRROT = ["scr0", "scr1", "Lp0", "Lp1"]
                        for half in range(2):
                            hbank = [bank(f"h{i}") for i in range(4)]
                            for ii in range(4):
                                i = 4 * half + ii
                                nc.tensor.matmul(
                                    hbank[ii][:], wc1a_s[i][:], senc[i][:, sl], start=True, stop=False
                                )
                            for j in range(N):
                                iis = [ii for ii in range(4) if 4 * half + ii != j]
                                prods = {}
                                for idx, ii in enumerate(iis):
                                    i = 4 * half + ii
                                    scr = bank(SCRROT[scnt[0] % 4])
                                    scnt[0] += 1
                                    g = i % 4
                                    nc.tensor.matmul(
                                        scr[:],
                                        selbc_s[32 * g : 32 * g + 32, j * HID : (j + 1) * HID],
                                        At[i // 4][32 * g : 32 * g + 32, :],
                                        start=True,
                                        stop=True,
                                        tile_position=(32 * g, 0),
                                    )
                                    ebs = wp.tile([HID, CS], bf16, tag=f"ebs{idx}", name=f"ebs{idx}")
                                    if scnt[0] % 4 == 0:
                                        nc.vector.tensor_copy(ebs[:], scr[:])
                                    else:
                                        nc.scalar.activation(ebs[:], scr[:], COPY)
                                    prod = wp.tile([HID, CS], bf16, tag=f"prod{idx}", name=f"prod{idx}")
                                    eng = nc.vector if scnt[0] % 4 == 2 else nc.gpsimd
                                    eng.tensor_tensor(prod[:], ebs[:], Vt[j][:, sl], MULT)
                                    prods[ii] = prod
                                for ii in iis:
                                    i = 4 * half + ii
                                    last_j = N - 1 if i != N - 1 else N - 2
                                    nc.tensor.matmul(
                                        hbank[ii][:],
                                        wc1b_s[i][:],
                                        prods[ii][:],
                                        start=False,
                                        stop=(j == last_j),
                                    )
                            # critic head: relu, wc2, bias, DMA out
                            for ii in range(4):
                                i = 4 * half + ii
                                h1 = wp.tile([HID, CS], bf16, tag="h1s", name="h1s")
                                nc.scalar.activation(h1[:], hbank[ii][:], RELU, bias=bC1_s[i][:])
                                nc.tensor.matmul(
                                    hbank[ii][:ADIM, :], wc2_s[i][:], h1[:], start=True, stop=True
                                )
                                aq = wp.tile([ADIM, CS], f32, tag="aq", name="aq")
                                nc.scalar.activation(aq[:], hbank[ii][:ADIM, :], IDENT, bias=bC2_s[i][:])
                                nc.sync.dma_start(out=q8[i, :, c0 : c0 + CS], in_=aq[:])
    if split:
        split_multi_waits(nc)
    return nc


def split_multi_waits(nc):
    """The 64B ISA instruction structs carry exactly ONE sync-wait slot.
    Tile emits instructions with several waits; walrus rejects them
    ("Too many sync wait commands").  Hoist all but one wait of each
    instruction onto a chain of same-engine NoOps placed directly before
    it in the instruction stream (queue-level stall, no pipe flush)."""
    import concourse.mybir as mybir

    nid = [0]
    for f in nc.m.functions:
        for blk in f.blocks:
            il = blk.instructions
            i = 0
            while i < len(il):
                inst = il[i]
                si = inst.sync_info
                if si is not None and si.on_wait and len(si.on_wait) > 1:
                    waits = list(si.on_wait)
                    extra, keep = waits[:-1], waits[-1:]
                    si.on_wait = keep
                    for w in extra:
                        nid[0] += 1
                        nop = mybir.InstNoOp(name=f"W-split-{nid[0]}", ins=[], outs=[])
                        nop.engine = inst.engine
                        nop.sync_info = mybir.SyncInfo(on_wait=[w], on_update=[])
                        il.insert(i, nop)
                        i += 1
                i += 1
    return nc


def host_prep(states, actions, We, be, Ws, bs, Wk, Wq, Wv, bv, Wc1, bc1, Wc2, bc2):
    """Pack/cast all tensors host-side. Returns (per_core_fn, acs)."""
    f32 = np.float32

    def bf(x):
        return np.ascontiguousarray(x, dtype=BF16)

    acs = np.argmax(actions, axis=-1)  # [N, B] (matches reference tie-breaking)

    # merged head weights: [h, k*AD+d]
    wk_m = np.concatenate([Wk[k] for k in range(HEADS)], axis=1)
    wq_m = np.concatenate([Wq[k] for k in range(HEADS)], axis=1)
    wv_m = np.concatenate([Wv[k] for k in range(HEADS)], axis=1)
    bv_m = np.concatenate([bv[k] for k in range(HEADS)], axis=0)  # [128]

    # onesred slice j: within-slice col (4j+k) <- rows (32k..32k+32)
    onesred = np.zeros((HID, N * 32), f32)
    for j in range(N):
        for k in range(HEADS):
            onesred[32 * k : 32 * (k + 1), 32 * j + 4 * j + k] = 1.0
    # seld32[i] (each 32-row block identical): col k <- rows (4j+k) j!=i;
    # cols 4..31 all-ones (keeps unused D rows finite/positive)
    blk = np.zeros((32, 32), f32)
    blk[:, 4:] = 1.0
    seld32 = np.tile(blk[None], (N, 4, 1))
    for i in range(N):
        for j in range(N):
            if j != i:
                for k in range(HEADS):
                    for g in range(4):
                        seld32[i, 32 * g + 4 * j + k, k] = 1.0
    # selrep2: col (32g+4j+k) <- row (32g+k) (within-bank broadcast)
    selrep2 = np.zeros((HID, HID), f32)
    for g in range(4):
        for j in range(N):
            for k in range(HEADS):
                selrep2[32 * g + k, 32 * g + 4 * j + k] = 1.0
    # selbc (each 32-row block identical): block-row (4j+k) -> cols (32k+d)
    selbc = np.zeros((HID, N * HID), f32)
    for g in range(4):
        for j in range(N):
            for k in range(HEADS):
                selbc[32 * g + 4 * j + k, HID * j + 32 * k : HID * j + 32 * (k + 1)] = 1.0

    shared = {
        "we1": bf(We[:, :SDIM, :]),
        "we2": bf(We[:, SDIM:, :]),
        "ws": bf(Ws),
        "wk": bf(wk_m),
        "wq": bf(wq_m),
        "wv": bf(wv_m),
        "wc1a": bf(Wc1[:, :HID, :]),
        "wc1b": bf(Wc1[:, HID:, :]),
        "wc2": bf(Wc2),
        "onesred": bf(onesred),
        "seld32": bf(seld32),
        "selrep2": bf(selrep2),
        "selbc": bf(selbc),
        "bE": np.ascontiguousarray(be[..., None], f32),
        "bS": np.ascontiguousarray(bs[..., None], f32),
        "bV": np.ascontiguousarray(bv_m[..., None], f32),
        "bC1": np.ascontiguousarray(bc1[..., None], f32),
        "bC2": np.ascontiguousarray(bc2[..., None], f32),
    }
    sT_full = bf(states.transpose(0, 2, 1))  # [N, 128, B]
    aT_full = bf(actions.transpose(0, 2, 1))  # [N, 16, B]

    def core_inputs(c, bsh):
        lo = c * bsh
        return dict(
            shared,
            sT=np.ascontiguousarray(sT_full[:, :, lo : lo + bsh]),
            aT=np.ascontiguousarray(aT_full[:, :, lo : lo + bsh]),
        )

    return core_inputs, acs


def kernel(**inputs):
    from concourse.bass_utils import run_bass_kernel_spmd

    nc = build_nc(BSH)
    core_inputs, acs = host_prep(**inputs)
    in_maps = [core_inputs(c, BSH) for c in range(NCORES)]
    res = run_bass_kernel_spmd(nc, in_maps, list(range(NCORES))).results
    out = np.empty((N, B, 1), np.float32)
    for c in range(NCORES):
        aq = res[c]["q8"]  # [N, 16, BSH]
        sl = slice(c * BSH, (c + 1) * BSH)
        out[:, sl, 0] = np.take_along_axis(aq, acs[:, None, sl], axis=1)[:, 0, :]
    return out


# revision 17
# speedup vs baseline: 1.0386x; 1.0386x over previous
"""Trainium2 Bass kernel for the AttentionCritic problem.

Strategy (pure data-parallel over batch, 8 cores), V2:
  - Host: transpose states/actions to feature-major, cast to bf16, pack
    per-head weights into merged [128,128] matrices, precompute the
    argmax one-hot selector on host, build small 0/1 selector matrices
    for PE-based partition reductions/broadcasts.
  - Device (per core, batch shard 4096, feature-major layout
    [feature_on_partitions, batch_on_free]):
      * dense encoders / K,Q,V / critic on TensorE (bf16), n-outer over
        superchunks so weight loads (LDWEIGHTS) amortize,
      * logits for 4 agents packed per PSUM bank; L-reduce uses one
        shared ones[128,4] lhsT writing 4-row regions (one LDW total),
      * exp on ScalarE over packed [128,S] banks (2 per chunk),
      * denominators for all 8 agents in one [32,S] bank; one
        reciprocal_approx_fast; normalization multiplier broadcast via
        one fp32r matmul per bank; A = E*M on Pool,
      * numerator: per (i,j) broadcast matmul (selbc_j lhsT, j-outer so
        LDW amortizes) -> PSUM, multiply with V_j on Pool/Vector
        reading PSUM directly -> bf16 SBUF product,
      * j-accumulation fused into the critic: h1_i = wc1a_i@senc_i +
        sum_j wc1b_i@(A_ij*V_j) accumulated in PSUM (linearity),
      * critic head relu / output bias on ScalarE, DMA out.
"""

import sys

sys.path.insert(0, "/opt/trn_rl_repo")

import numpy as np
import ml_dtypes

N, B, SDIM, ADIM, HID, HEADS = 8, 32768, 128, 16, 128, 4
AD = HID // HEADS
IDIM = SDIM + ADIM
NCORES = 8
BSH = B // NCORES
BF16 = ml_dtypes.bfloat16
INV_SQRT_AD = 1.0 / np.sqrt(AD).astype(np.float32)


def build_nc(bsh, SC=1024, CS=512, split=True):
    """Build the Bass module for one core processing a batch shard of bsh.

    SC: superchunk (dense n-outer granularity; senc/K/Q/V persist per SC)
    CS: chunk (PSUM bank free size; all attention tiles are [*, CS])
    """
    import concourse.bass as bass
    import concourse.mybir as mybir
    from concourse.tile import TileContext

    f32 = mybir.dt.float32
    f32r = mybir.dt.float32r
    bf16 = mybir.dt.bfloat16
    MULT = mybir.AluOpType.mult
    ADD = mybir.AluOpType.add
    MAX = mybir.AluOpType.max
    COPY = mybir.ActivationFunctionType.Copy
    RELU = mybir.ActivationFunctionType.Relu
    EXP = mybir.ActivationFunctionType.Exp
    IDENT = mybir.ActivationFunctionType.Identity

    SC = min(SC, bsh)
    CS = min(CS, SC)
    n_sc = bsh // SC
    n_cs = SC // CS

    nc = bass.Bass()

    # ---- DRAM parameters ----
    dp = nc.declare_dram_parameter
    sT = dp("sT", [N, SDIM, bsh], bf16, isOutput=False)
    aT = dp("aT", [N, ADIM, bsh], bf16, isOutput=False)
    we1 = dp("we1", [N, SDIM, HID], bf16, isOutput=False)
    we2 = dp("we2", [N, ADIM, HID], bf16, isOutput=False)
    ws = dp("ws", [N, SDIM, HID], bf16, isOutput=False)
    wk = dp("wk", [HID, HID], bf16, isOutput=False)
    wq = dp("wq", [HID, HID], bf16, isOutput=False)
    wv = dp("wv", [HID, HID], bf16, isOutput=False)
    wc1a = dp("wc1a", [N, HID, HID], bf16, isOutput=False)
    wc1b = dp("wc1b", [N, HID, HID], bf16, isOutput=False)
    wc2 = dp("wc2", [N, HID, ADIM], bf16, isOutput=False)
    onesred = dp("onesred", [HID, N * 32], bf16, isOutput=False)  # per-j L-reduce
    seld32 = dp("seld32", [N, HID, 32], bf16, isOutput=False)  # E(j,k),j!=i -> k
    selrep2 = dp("selrep2", [HID, HID], bf16, isOutput=False)  # Dinv(g,k)->(g,j,k)
    selbc = dp("selbc", [HID, N * HID], bf16, isOutput=False)  # (j,k)->(k,d)
    bE = dp("bE", [N, HID, 1], f32, isOutput=False)
    bS = dp("bS", [N, HID, 1], f32, isOutput=False)
    bV = dp("bV", [HID, 1], f32, isOutput=False)
    bC1 = dp("bC1", [N, HID, 1], f32, isOutput=False)
    bC2 = dp("bC2", [N, ADIM, 1], f32, isOutput=False)
    q8 = dp("q8", [N, ADIM, bsh], f32, isOutput=True)

    with TileContext(nc) as tc:
        with tc.tile_pool(name="const", bufs=1) as cp:

            def cload(ap, shape, dt, tag):
                t = cp.tile(shape, dt, tag=tag)
                nc.sync.dma_start(out=t[:], in_=ap)
                return t

            we1_s = [cload(we1[n], [SDIM, HID], bf16, f"we1{n}") for n in range(N)]
            we2_s = [cload(we2[n], [ADIM, HID], bf16, f"we2{n}") for n in range(N)]
            ws_s = [cload(ws[n], [SDIM, HID], bf16, f"ws{n}") for n in range(N)]
            wk_s = cload(wk[:], [HID, HID], bf16, "wk")
            wq_s = cload(wq[:], [HID, HID], bf16, "wq")
            wv_s = cload(wv[:], [HID, HID], bf16, "wv")
            wc1a_s = [cload(wc1a[n], [HID, HID], bf16, f"wc1a{n}") for n in range(N)]
            wc1b_s = [cload(wc1b[n], [HID, HID], bf16, f"wc1b{n}") for n in range(N)]
            wc2_s = [cload(wc2[n], [HID, ADIM], bf16, f"wc2{n}") for n in range(N)]
            onesred_s = cload(onesred[:], [HID, N * 32], bf16, "onesred")
            seld32_s = [cload(seld32[n], [HID, 32], bf16, f"seld32{n}") for n in range(N)]
            selrep2_s = cload(selrep2[:], [HID, HID], bf16, "selrep2")
            selbc_s = cload(selbc[:], [HID, N * HID], bf16, "selbc")
            bE_s = [cload(bE[n], [HID, 1], f32, f"bE{n}") for n in range(N)]
            bS_s = [cload(bS[n], [HID, 1], f32, f"bS{n}") for n in range(N)]
            bV_s = cload(bV[:], [HID, 1], f32, "bV")
            bC1_s = [cload(bC1[n], [HID, 1], f32, f"bC1{n}") for n in range(N)]
            bC2_s = [cload(bC2[n], [ADIM, 1], f32, f"bC2{n}") for n in range(N)]

            with (
                tc.tile_pool(name="persist", bufs=1) as pp,
                tc.tile_pool(name="work", bufs=2) as wp,
                tc.tile_pool(name="prp", bufs=8) as prp,
                tc.tile_pool(name="psum", bufs=1, space="PSUM") as qp,
            ):
                # 8 PSUM banks, one tag each.  Dense phase reuses them so
                # every region is written (start=True) before being read.
                def bank(tag):
                    return qp.tile([HID, CS], f32, tag=tag, name=tag)

                BTAGS = ["Lp0", "Lp1", "scr0", "scr1", "h0", "h1", "h2", "h3"]

                for sc in range(n_sc):
                    sc0 = sc * SC
                    senc = [pp.tile([HID, SC], bf16, tag=f"senc{n}", name=f"senc{n}") for n in range(N)]
                    Kt = [pp.tile([HID, SC], bf16, tag=f"K{n}", name=f"K{n}") for n in range(N)]
                    Qt = [pp.tile([HID, SC], bf16, tag=f"Q{n}", name=f"Q{n}") for n in range(N)]
                    Vt = [pp.tile([HID, SC], bf16, tag=f"V{n}", name=f"V{n}") for n in range(N)]

                    # ---------- dense phase (n-outer, LDW amortized) ------
                    for n in range(N):
                        for c in range(n_cs):
                            c0 = sc0 + c * CS
                            sl = slice(c * CS, (c + 1) * CS)
                            st = wp.tile([SDIM, CS], bf16, tag="st", name="st")
                            at = wp.tile([ADIM, CS], bf16, tag="at", name="at")
                            nc.sync.dma_start(out=st[:], in_=sT[n, :, c0 : c0 + CS])
                            nc.sync.dma_start(out=at[:], in_=aT[n, :, c0 : c0 + CS])
                            pe_ = bank(BTAGS[(2 * c) % 8])
                            po_ = bank(BTAGS[(2 * c + 1) % 8])
                            # saenc = relu(we1@st + we2@at + bE)
                            nc.tensor.matmul(pe_[:], we1_s[n][:], st[:], start=True, stop=False)
                            nc.tensor.matmul(pe_[:], we2_s[n][:], at[:], start=False, stop=True)
                            saenc = wp.tile([HID, CS], bf16, tag="saenc", name="saenc")
                            nc.scalar.activation(saenc[:], pe_[:], RELU, bias=bE_s[n][:])
                            # senc = relu(ws@st + bS)
                            nc.tensor.matmul(po_[:], ws_s[n][:], st[:], start=True, stop=True)
                            nc.scalar.activation(senc[n][:, sl], po_[:], RELU, bias=bS_s[n][:])
                            # K = wk@saenc ; Q = wq@senc ; V = relu(wv@saenc+bV)
                            pk = bank(BTAGS[(2 * c + 2) % 8])
                            nc.tensor.matmul(pk[:], wk_s[:], saenc[:], start=True, stop=True)
                            nc.scalar.activation(Kt[n][:, sl], pk[:], COPY)
                            pq = bank(BTAGS[(2 * c + 3) % 8])
                            nc.tensor.matmul(pq[:], wq_s[:], senc[n][:, sl], start=True, stop=True)
                            nc.scalar.activation(Qt[n][:, sl], pq[:], COPY)
                            pv = bank(BTAGS[(2 * c + 4) % 8])
                            nc.tensor.matmul(pv[:], wv_s[:], saenc[:], start=True, stop=True)
                            nc.scalar.activation(Vt[n][:, sl], pv[:], RELU, bias=bV_s[:])

                    # ---------- attention + critic, per chunk -------------
                    for c in range(n_cs):
                        c0 = sc0 + c * CS
                        sl = slice(c * CS, (c + 1) * CS)
                        Lp = [bank("Lp0"), bank("Lp1")]
                        # logits: pair (i,j) accumulates rows (4j+k) of the
                        # 32-row region 32*(i%4) of bank i//4; self rows stay
                        # exactly zero (every lhsT slice is 0 there)
                        for j in range(N):
                            for i in range(N):
                                if i == j:
                                    continue
                                pr = prp.tile([HID, CS], bf16, tag="pr", name="pr")
                                nc.vector.tensor_tensor(pr[:], Qt[i][:, sl], Kt[j][:, sl], MULT)
                                g = i % 4
                                nc.tensor.matmul(
                                    Lp[i // 4][32 * g : 32 * g + 32, :],
                                    onesred_s[:, 32 * j : 32 * j + 32],
                                    pr[:],
                                    start=(j == (1 if i == 0 else 0)),
                                    stop=(j == (N - 2 if i == N - 1 else N - 1)),
                                    tile_position=(0, 32 * g),
                                    skip_group_check=True,
                                )
                        # E = exp(L/sqrt(d)) (bf16); self rows exp(0)=1, excluded by seld
                        Et = [wp.tile([HID, CS], bf16, tag=f"E{b}", name=f"E{b}") for b in range(2)]
                        for b in range(2):
                            nc.scalar.activation(Et[b][:], Lp[b][:], EXP, scale=float(INV_SQRT_AD))
                        # denominators: D_i head k at row 32*(i%4)+k of bank i//4
                        # (cols 4..31 of seld32 are all-ones fillers so the rest
                        # of each region holds finite positive sums)
                        Dp = [bank("scr0"), bank("scr1")]
                        for i in range(N):
                            g = i % 4
                            nc.tensor.matmul(
                                Dp[i // 4][32 * g : 32 * g + 32, :],
                                seld32_s[i][32 * g : 32 * g + 32, :],
                                Et[i // 4][32 * g : 32 * g + 32, :],
                                start=True,
                                stop=True,
                                tile_position=(32 * g, 32 * g),
                                skip_group_check=True,
                            )
                        Dinv = [wp.tile([HID, CS], f32, tag=f"Dinv{b}", name=f"Dinv{b}") for b in range(2)]
                        Dinvb = [wp.tile([HID, CS], bf16, tag=f"Dinvb{b}", name=f"Dinvb{b}") for b in range(2)]
                        for b in range(2):
                            nc.vector.reciprocal(Dinv[b][:], Dp[b][:])
                            nc.vector.tensor_copy(Dinvb[b][:], Dinv[b][:])
                        # A = E * broadcast(Dinv) per bank (fp32r matmul + Pool)
                        At = [wp.tile([HID, CS], bf16, tag=f"A{b}", name=f"A{b}") for b in range(2)]
                        for b in range(2):
                            Mp = bank("scr1" if b == 0 else "scr0")
                            nc.tensor.matmul(
                                Mp[:],
                                selrep2_s[:],
                                Dinvb[b][:],
                                start=True,
                                stop=True,
                            )
                            nc.vector.tensor_tensor(At[b][:], Et[b][:], Mp[:], MULT)
                        # numerator + critic h1, i-halves x j-outer
                        scnt = [0]
                        SCRROT = ["scr0", "scr1", "Lp0", "Lp1"]
                        for half in range(2):
                            hbank = [bank(f"h{i}") for i in range(4)]
                            for ii in range(4):
                                i = 4 * half + ii
                                nc.tensor.matmul(
                                    hbank[ii][:], wc1a_s[i][:], senc[i][:, sl], start=True, stop=False
                                )
                            for j in range(N):
                                iis = [ii for ii in range(4) if 4 * half + ii != j]
                                prods = {}
                                for idx, ii in enumerate(iis):
                                    i = 4 * half + ii
                                    scr = bank(SCRROT[scnt[0] % 4])
                                    scnt[0] += 1
                                    g = i % 4
                                    nc.tensor.matmul(
                                        scr[:],
                                        selbc_s[32 * g : 32 * g + 32, j * HID : (j + 1) * HID],
                                        At[i // 4][32 * g : 32 * g + 32, :],
                                        start=True,
                                        stop=True,
                                        tile_position=(32 * g, 0),
                                    )
                                    ebs = wp.tile([HID, CS], bf16, tag=f"ebs{idx}", name=f"ebs{idx}")
                                    if scnt[0] % 4 == 0:
                                        nc.vector.tensor_copy(ebs[:], scr[:])
                                    else:
                                        nc.scalar.activation(ebs[:], scr[:], COPY)
                                    prod = wp.tile([HID, CS], bf16, tag=f"prod{idx}", name=f"prod{idx}")
                                    eng = nc.gpsimd if scnt[0] % 4 == 1 else nc.vector
                                    eng.tensor_tensor(prod[:], ebs[:], Vt[j][:, sl], MULT)
                                    prods[ii] = prod
                                for ii in iis:
                                    i = 4 * half + ii
                                    last_j = N - 1 if i != N - 1 else N - 2
                                    nc.tensor.matmul(
                                        hbank[ii][:],
                                        wc1b_s[i][:],
                                        prods[ii][:],
                                        start=False,
                                        stop=(j == last_j),
                                    )
                            # critic head: relu, wc2, bias, DMA out
                            for ii in range(4):
                                i = 4 * half + ii
                                h1 = wp.tile([HID, CS], bf16, tag="h1s", name="h1s")
                                nc.scalar.activation(h1[:], hbank[ii][:], RELU, bias=bC1_s[i][:])
                                nc.tensor.matmul(
                                    hbank[ii][:ADIM, :], wc2_s[i][:], h1[:], start=True, stop=True
                                )
                                aq = wp.tile([ADIM, CS], f32, tag="aq", name="aq")
                                nc.scalar.activation(aq[:], hbank[ii][:ADIM, :], IDENT, bias=bC2_s[i][:])
                                nc.sync.dma_start(out=q8[i, :, c0 : c0 + CS], in_=aq[:])
    if split:
        split_multi_waits(nc)
    return nc


def split_multi_waits(nc):
    """The 64B ISA instruction structs carry exactly ONE sync-wait slot.
    Tile emits instructions with several waits; walrus rejects them
    ("Too many sync wait commands").  Hoist all but one wait of each
    instruction onto a chain of same-engine NoOps placed directly before
    it in the instruction stream (queue-level stall, no pipe flush)."""
    import concourse.mybir as mybir

    nid = [0]
    for f in nc.m.functions:
        for blk in f.blocks:
            il = blk.instructions
            i = 0
            while i < len(il):
                inst = il[i]
                si = inst.sync_info
                if si is not None and si.on_wait and len(si.on_wait) > 1:
                    waits = list(si.on_wait)
                    extra, keep = waits[:-1], waits[-1:]
                    si.on_wait = keep
                    for w in extra:
                        nid[0] += 1
                        nop = mybir.InstNoOp(name=f"W-split-{nid[0]}", ins=[], outs=[])
                        nop.engine = inst.engine
                        nop.sync_info = mybir.SyncInfo(on_wait=[w], on_update=[])
                        il.insert(i, nop)
                        i += 1
                i += 1
    return nc


def host_prep(states, actions, We, be, Ws, bs, Wk, Wq, Wv, bv, Wc1, bc1, Wc2, bc2):
    """Pack/cast all tensors host-side. Returns (per_core_fn, acs)."""
    f32 = np.float32

    def bf(x):
        return np.ascontiguousarray(x, dtype=BF16)

    acs = np.argmax(actions, axis=-1)  # [N, B] (matches reference tie-breaking)

    # merged head weights: [h, k*AD+d]
    wk_m = np.concatenate([Wk[k] for k in range(HEADS)], axis=1)
    wq_m = np.concatenate([Wq[k] for k in range(HEADS)], axis=1)
    wv_m = np.concatenate([Wv[k] for k in range(HEADS)], axis=1)
    bv_m = np.concatenate([bv[k] for k in range(HEADS)], axis=0)  # [128]

    # onesred slice j: within-slice col (4j+k) <- rows (32k..32k+32)
    onesred = np.zeros((HID, N * 32), f32)
    for j in range(N):
        for k in range(HEADS):
            onesred[32 * k : 32 * (k + 1), 32 * j + 4 * j + k] = 1.0
    # seld32[i] (each 32-row block identical): col k <- rows (4j+k) j!=i;
    # cols 4..31 all-ones (keeps unused D rows finite/positive)
    blk = np.zeros((32, 32), f32)
    blk[:, 4:] = 1.0
    seld32 = np.tile(blk[None], (N, 4, 1))
    for i in range(N):
        for j in range(N):
            if j != i:
                for k in range(HEADS):
                    for g in range(4):
                        seld32[i, 32 * g + 4 * j + k, k] = 1.0
    # selrep2: col (32g+4j+k) <- row (32g+k) (within-bank broadcast)
    selrep2 = np.zeros((HID, HID), f32)
    for g in range(4):
        for j in range(N):
            for k in range(HEADS):
                selrep2[32 * g + k, 32 * g + 4 * j + k] = 1.0
    # selbc (each 32-row block identical): block-row (4j+k) -> cols (32k+d)
    selbc = np.zeros((HID, N * HID), f32)
    for g in range(4):
        for j in range(N):
            for k in range(HEADS):
                selbc[32 * g + 4 * j + k, HID * j + 32 * k : HID * j + 32 * (k + 1)] = 1.0

    shared = {
        "we1": bf(We[:, :SDIM, :]),
        "we2": bf(We[:, SDIM:, :]),
        "ws": bf(Ws),
        "wk": bf(wk_m),
        "wq": bf(wq_m),
        "wv": bf(wv_m),
        "wc1a": bf(Wc1[:, :HID, :]),
        "wc1b": bf(Wc1[:, HID:, :]),
        "wc2": bf(Wc2),
        "onesred": bf(onesred),
        "seld32": bf(seld32),
        "selrep2": bf(selrep2),
        "selbc": bf(selbc),
        "bE": np.ascontiguousarray(be[..., None], f32),
        "bS": np.ascontiguousarray(bs[..., None], f32),
        "bV": np.ascontiguousarray(bv_m[..., None], f32),
        "bC1": np.ascontiguousarray(bc1[..., None], f32),
        "bC2": np.ascontiguousarray(bc2[..., None], f32),
    }
    sT_full = bf(states.transpose(0, 2, 1))  # [N, 128, B]
    aT_full = bf(actions.transpose(0, 2, 1))  # [N, 16, B]

    def core_inputs(c, bsh):
        lo = c * bsh
        return dict(
            shared,
            sT=np.ascontiguousarray(sT_full[:, :, lo : lo + bsh]),
            aT=np.ascontiguousarray(aT_full[:, :, lo : lo + bsh]),
        )

    return core_inputs, acs


def kernel(**inputs):
    from concourse.bass_utils import run_bass_kernel_spmd

    nc = build_nc(BSH)
    core_inputs, acs = host_prep(**inputs)
    in_maps = [core_inputs(c, BSH) for c in range(NCORES)]
    res = run_bass_kernel_spmd(nc, in_maps, list(range(NCORES))).results
    out = np.empty((N, B, 1), np.float32)
    for c in range(NCORES):
        aq = res[c]["q8"]  # [N, 16, BSH]
        sl = slice(c * BSH, (c + 1) * BSH)
        out[:, sl, 0] = np.take_along_axis(aq, acs[:, None, sl], axis=1)[:, 0, :]
    return out
